# revision 13
# baseline (speedup 1.0000x reference)
"""GNN message-passing ConvNet layer on 8 TRN2 NeuronCores (Bass/Tile).

Computes, for x [B=4, N=4096, D=128], adj_mat [B, N, N] (0/1 floats),
U [D, D]:
    mask = (adj_mat > 0)
    deg[b, i] = sum_j adj_mat[b, j, i]
    agg[b, i, :] = sum_j mask[b, j, i] * x[b, j, :]
    out = relu((agg @ U) / deg[..., None])

Sharding: core c handles batch c//2 and destination-node half c%2 (the
column slice adj[b, :, i0:i0+2048]) — no collectives, identical per-core
work.

Per-core kernel (v2 pipeline; the v1 at 55-59us lost ~10us to a HAM
re-throttle caused by the x_lo tail crawling in on the SWDGE queue, and
~8us to an end-of-kernel semaphore chain over ~55 tiles):
  - adj is 0/1 so it is packed host-side to float8e4 (exact) — 8 MiB per
    core instead of 32 MiB. DRAM layout [128p][round][jtile][i] makes
    every DMA fully contiguous per partition.
  - x is split host-side into an fp8 hi/lo pair (x ~= hi + lo at ~bf16
    accuracy), interleaved by 256-row pair in consumption order in ONE
    tensor, and loaded over the two HWDGE queues (head on sync, tail on
    scalar) so no matmul ever waits on the slow SWDGE path.
  - Per 256-row j-pair, two DoubleRow fp8 matmuls (lo then hi) stream
    the same adj tile and accumulate into a SINGLE PSUM bank — there is
    no separate A/B combine, no degree row, no reciprocal, no partition
    broadcast. relu(z)/deg == relu(z/deg) for deg>0, so the 1/deg
    column scale is applied on the host after the gather (deg is
    computed host-side from adj; the HW returns relu(agg @ U) only).
  - adj chunks alternate between the sync and scalar HWDGE queues
    (~175 GB/s each when both active, ~350 aggregate vs the 358 GB/s
    per-core HBM cap), in consumption order per queue.
  - DoubleRow contracts 256 rows/output-column at 1 col/cycle, so the
    agg stream is ~66k PE cycles (~27.5 us warm) — the PE floor for the
    hi+lo algorithm; the DMA floor (~9.5 MiB at ~350 GB/s) is ~27.5 us
    too, so the two overlap almost exactly.
  - A handful of warmup matmuls on a zeroed scratch tile cover the
    ~3.5 us DMA ramp and flip the HAM clock-gate to 2.4 GHz just before
    real data lands; the real stream then keeps the PE saturated so the
    gate never drops back.
  - Per-round tail (emitted one chunk into the next round): DVE copies
    the round's PSUM to SBUF as f32r, one U-matmul (stationary U f32r,
    moving the copied sum) lands relu-input in PSUM, ACT applies ReLU
    and casts to bf16, and the [e, i] tile is stored (host transposes,
    upcasts, and divides by deg). Rounds 0-2 store on the SWDGE queue;
    the final round is split in column halves across sync+scalar so the
    end-of-kernel drain pipelines.
"""

import os
import sys

for _p in ("/opt/trn_rl_repo",):
    if _p not in sys.path and os.path.isdir(_p):
        sys.path.insert(0, _p)

from contextlib import ExitStack

import numpy as np
import ml_dtypes

B, N, D = 4, 4096, 128
P = 128
N_CORES = 8
W = 512                 # destination columns per round (one PSUM bank)
I_CORE = N // 2         # destination columns per core
N_ROUNDS = I_CORE // W  # 4
NJT = N // P            # 32 j-tiles of 128 rows
NPAIR = NJT // 2        # 16 DoubleRow pairs of 256 rows
XS_HEAD = 2             # x pairs loaded on sync ahead of the adj stream
N_WARM = 7              # cold warmup matmuls bridging the DMA ramp

_PROG = None


def _build_program():
    from concourse import mybir, tile, bacc

    f32 = mybir.dt.float32
    f32r = mybir.dt.float32r
    bf16 = mybir.dt.bfloat16
    fp8 = mybir.dt.float8e4
    DR = mybir.MatmulPerfMode.DoubleRow
    RELU = mybir.ActivationFunctionType.Relu

    nc = bacc.Bacc(
        "TRN2",
        target_bir_lowering=False,
        debug=False,
        enable_asserts=False,
        num_devices=N_CORES,
    )
    # [p][round][jtile][i] — per partition each round's block is 16 KiB
    # contiguous, so every chunk DMA is clean per-partition runs.
    adj_d = nc.dram_tensor("adj_p", [P, N_ROUNDS, NJT, W], fp8, kind="ExternalInput")
    # x hi/lo interleaved by pair in consumption order:
    # [p][pair][slot lo|hi][jt-in-pair][d]
    x2_d = nc.dram_tensor("x2_p", [P, NPAIR, 2, 2, D], fp8, kind="ExternalInput")
    u_d = nc.dram_tensor("U", [D, D], bf16, kind="ExternalInput")
    # output [e, i_core] bf16, UNSCALED relu(agg@U); host transposes,
    # upcasts and divides by deg.
    out_d = nc.dram_tensor("out_t", [P, I_CORE], bf16, kind="ExternalOutput")

    # (pairs, dma engine name) per chunk, per round; consumption order.
    # Round 0 ramps with small chunks on sync (its first item, so the
    # first matmul's data lands ~3.3us in); pairs 12-15 of round 0 and
    # the second half of every later round ride the scalar queue, which
    # first carries the x head + U + x tail.
    CHUNKS = [
        [(1, "sync"), (1, "sync"), (2, "sync"), (4, "sync"), (4, "sync"), (4, "scalar")],
        [(8, "sync"), (8, "scalar")],
        [(8, "sync"), (8, "scalar")],
        [(8, "sync"), (8, "scalar")],
    ]

    with tile.TileContext(nc, trace_sim=False) as tc, ExitStack() as ctx:
        const_pool = ctx.enter_context(tc.tile_pool(name="const", bufs=1))
        adj_pool = ctx.enter_context(tc.tile_pool(name="adj", bufs=8))
        sum_pool = ctx.enter_context(tc.tile_pool(name="sum", bufs=2))
        out_pool = ctx.enter_context(tc.tile_pool(name="out", bufs=2))
        ps_c = ctx.enter_context(tc.tile_pool(name="ps_c", bufs=2, space="PSUM"))
        ps_o = ctx.enter_context(tc.tile_pool(name="ps_o", bufs=2, space="PSUM"))
        ps_w = ctx.enter_context(tc.tile_pool(name="ps_w", bufs=1, space="PSUM"))

        # --- warmup: dummy matmuls on a zeroed tile keep the PE busy
        # through the DMA ramp and flip the HAM clock-gate to 2.4 GHz
        # just as the first real chunk's semaphore fires. ---
        warm_sb = const_pool.tile([P, 2, W], fp8)
        nc.vector.memset(warm_sb[:], 0.0)
        warm_ps = ps_w.tile([P, W], f32, tag="warm")
        for _ in range(N_WARM):
            nc.tensor.matmul(
                warm_ps[:],
                warm_sb[:, :, 0:D],
                warm_sb[:],
                start=True,
                stop=True,
                perf_mode=DR,
            )

        # --- constant loads. The x head is one small tile and the tail
        # is PER-PAIR tiles/DMAs in consumption order, all on the scalar
        # queue (sync's first item is adj chunk 0). Fine granularity
        # matters: each pair's matmuls wait only on that pair's DMA, so
        # the scheduler's hoisted waits line up with arrival order
        # (a single 896 KiB tail DMA gated the whole stream at ~17us). ---
        x2h_sb = const_pool.tile([P, XS_HEAD, 2, 2, D], fp8)
        nc.scalar.dma_start(x2h_sb[:], x2_d[:, 0:XS_HEAD])
        u_sb = const_pool.tile([P, D], bf16)
        nc.scalar.dma_start(u_sb[:], u_d[:])
        xp_sb = [None] * XS_HEAD
        for pt in range(XS_HEAD, NPAIR):
            t = const_pool.tile([P, 1, 2, 2, D], fp8, tag=f"xp{pt}")
            nc.scalar.dma_start(t[:], x2_d[:, pt : pt + 1])
            xp_sb.append(t)

        def x_slice(pt, slot):
            if pt < XS_HEAD:
                return x2h_sb[:, pt, slot]
            return xp_sb[pt][:, 0, slot]

        def emit_tail(q, c_ps):
            """Tail of round q: copy the accumulated PSUM to SBUF as
            bf16, U-matmul, ReLU+bf16 cast, store. The last round is
            split in quarters alternating the two HWDGE queues so the
            final drain pipelines; earlier rounds store via SWDGE."""
            last = q == N_ROUNDS - 1
            split = 4 if last else 1
            ws = W // split
            o_ps = ps_o.tile([P, W], f32, tag="ops")
            for h in range(split):
                cs = slice(h * ws, (h + 1) * ws)
                c_sb = sum_pool.tile([P, ws], bf16, tag=f"csb{split}{h}")
                nc.vector.tensor_copy(c_sb[:], c_ps[:, cs])
                nc.tensor.matmul(
                    o_ps[:, cs], u_sb[:], c_sb[:], start=True, stop=True
                )
                out_sb = out_pool.tile([P, ws], bf16, tag=f"osb{split}{h}")
                nc.scalar.activation(out_sb[:], o_ps[:, cs], RELU)
                # Early rounds store on scalar (its in-flight adj bytes
                # keep flowing while the trigger waits on ACT); the last
                # round's quarters alternate queues to pipeline the drain.
                eng = (nc.scalar if h % 2 == 0 else nc.sync) if last else nc.scalar
                eng.dma_start(
                    out_d[:, q * W + h * ws : q * W + (h + 1) * ws], out_sb[:]
                )

        pending = None
        for q in range(N_ROUNDS):
            c_ps = ps_c.tile([P, W], f32, tag="c")
            chunk_list = CHUNKS[q]
            n_chunks = len(chunk_list)
            pt0 = 0
            for c, (cp, eng_name) in enumerate(chunk_list):
                adj_sb = adj_pool.tile([P, 2 * cp, W], fp8, tag="adj")
                getattr(nc, eng_name).dma_start(
                    adj_sb[:], adj_d[:, q, 2 * pt0 : 2 * (pt0 + cp), :]
                )
                for u in range(cp):
                    pt = pt0 + u
                    for slot in range(2):  # lo then hi
                        nc.tensor.matmul(
                            c_ps[:],
                            x_slice(pt, slot),
                            adj_sb[:, 2 * u : 2 * u + 2, :],
                            start=(c == 0 and u == 0 and slot == 0),
                            stop=(c == n_chunks - 1 and u == cp - 1 and slot == 1),
                            perf_mode=DR,
                        )
                pt0 += cp
                if pending is not None and c == 0:
                    emit_tail(*pending)
                    pending = None
            pending = (q, c_ps)
        emit_tail(*pending)

    nc.compile()
    return nc


def _get_program():
    global _PROG
    if _PROG is None:
        _PROG = _build_program()
    return _PROG


E4 = ml_dtypes.float8_e4m3


def _shard_inputs(x, adj_mat, U):
    # adj -> fp8 via bit trick: 0/1 exact (1.0 == 0x38 in e4m3).
    adj8 = (adj_mat != 0).astype(np.uint8) * np.uint8(0x38)
    x32 = np.asarray(x, dtype=np.float32)
    u16 = np.ascontiguousarray(U.astype(ml_dtypes.bfloat16))
    in_maps = []
    for c in range(N_CORES):
        b, half = c // 2, c % 2
        i0 = half * I_CORE
        a = adj8[b, :, i0 : i0 + I_CORE]  # [N, I_CORE] uint8
        # [t*128+p, q*512+i] -> [p, q, t, i]
        a = np.ascontiguousarray(
            a.reshape(NJT, P, N_ROUNDS, W).transpose(1, 2, 0, 3)
        ).view(E4)
        xb = x32[b]  # [N, D]
        xh = xb.astype(E4)
        xl = (xb - xh.astype(np.float32)).astype(E4)
        # [pt, jtp, p, d] -> [p, pt, slot, jtp, d]
        x2 = np.empty((P, NPAIR, 2, 2, D), dtype=E4)
        x2[:, :, 0] = xl.reshape(NPAIR, 2, P, D).transpose(2, 0, 1, 3)
        x2[:, :, 1] = xh.reshape(NPAIR, 2, P, D).transpose(2, 0, 1, 3)
        in_maps.append({"adj_p": a, "x2_p": x2, "U": u16})
    return in_maps


def _run(x, adj_mat, U, trace=False):
    from concourse.bass_utils import run_bass_kernel_spmd

    nc = _get_program()
    in_maps = _shard_inputs(x, adj_mat, U)
    res = run_bass_kernel_spmd(
        nc, in_maps, core_ids=list(range(N_CORES)), trace=trace
    )
    deg = np.asarray(adj_mat, dtype=np.float32).sum(axis=1)  # [B, N]
    out = np.empty((B, N, D), dtype=np.float32)
    for c in range(N_CORES):
        b, half = c // 2, c % 2
        i0 = half * I_CORE
        ot = res.results[c]["out_t"].astype(np.float32)  # [128 e, I_CORE]
        out[b, i0 : i0 + I_CORE, :] = (ot / deg[b, i0 : i0 + I_CORE][None, :]).T
    return out, res


def kernel(x, adj_mat, U):
    out, _ = _run(
        np.asarray(x, dtype=np.float32),
        np.asarray(adj_mat, dtype=np.float32),
        np.asarray(U, dtype=np.float32),
    )
    return out


# revision 15
# speedup vs baseline: 1.0782x; 1.0782x over previous
"""GNN message-passing ConvNet layer on 8 TRN2 NeuronCores (Bass/Tile).

Computes, for x [B=4, N=4096, D=128], adj_mat [B, N, N] (0/1 floats),
U [D, D]:
    mask = (adj_mat > 0)
    deg[b, i] = sum_j adj_mat[b, j, i]
    agg[b, i, :] = sum_j mask[b, j, i] * x[b, j, :]
    out = relu((agg @ U) / deg[..., None])

Sharding: core c handles batch c//2 and destination-node half c%2 (the
column slice adj[b, :, i0:i0+2048]) — no collectives, identical per-core
work.

Per-core kernel (v2 pipeline; the v1 at 55-59us lost ~10us to a HAM
re-throttle caused by the x_lo tail crawling in on the SWDGE queue, and
~8us to an end-of-kernel semaphore chain over ~55 tiles):
  - adj is 0/1 so it is packed host-side to float8e4 (exact) — 8 MiB per
    core instead of 32 MiB. DRAM layout [128p][round][jtile][i] makes
    every DMA fully contiguous per partition.
  - x is split host-side into an fp8 hi/lo pair (x ~= hi + lo at ~bf16
    accuracy), interleaved by 256-row pair in consumption order in ONE
    tensor, and loaded over the two HWDGE queues (head on sync, tail on
    scalar) so no matmul ever waits on the slow SWDGE path.
  - Per 256-row j-pair, two DoubleRow fp8 matmuls (lo then hi) stream
    the same adj tile and accumulate into a SINGLE PSUM bank — there is
    no separate A/B combine, no degree row, no reciprocal, no partition
    broadcast. relu(z)/deg == relu(z/deg) for deg>0, so the 1/deg
    column scale is applied on the host after the gather (deg is
    computed host-side from adj; the HW returns relu(agg @ U) only).
  - adj chunks alternate between the sync and scalar HWDGE queues
    (~175 GB/s each when both active, ~350 aggregate vs the 358 GB/s
    per-core HBM cap), in consumption order per queue.
  - DoubleRow contracts 256 rows/output-column at 1 col/cycle, so the
    agg stream is ~66k PE cycles (~27.5 us warm) — the PE floor for the
    hi+lo algorithm; the DMA floor (~9.5 MiB at ~350 GB/s) is ~27.5 us
    too, so the two overlap almost exactly.
  - A handful of warmup matmuls on a zeroed scratch tile cover the
    ~3.5 us DMA ramp and flip the HAM clock-gate to 2.4 GHz just before
    real data lands; the real stream then keeps the PE saturated so the
    gate never drops back.
  - Per-round tail (emitted one chunk into the next round): DVE copies
    the round's PSUM to SBUF as f32r, one U-matmul (stationary U f32r,
    moving the copied sum) lands relu-input in PSUM, ACT applies ReLU
    and casts to bf16, and the [e, i] tile is stored (host transposes,
    upcasts, and divides by deg). Rounds 0-2 store on the SWDGE queue;
    the final round is split in column halves across sync+scalar so the
    end-of-kernel drain pipelines.
"""

import os
import sys

for _p in ("/opt/trn_rl_repo",):
    if _p not in sys.path and os.path.isdir(_p):
        sys.path.insert(0, _p)

from contextlib import ExitStack

import numpy as np
import ml_dtypes

B, N, D = 4, 4096, 128
P = 128
N_CORES = 8
W = 512                 # destination columns per round (one PSUM bank)
I_CORE = N // 2         # destination columns per core
N_ROUNDS = I_CORE // W  # 4
NJT = N // P            # 32 j-tiles of 128 rows
NPAIR = NJT // 2        # 16 DoubleRow pairs of 256 rows
XS_HEAD = 2             # x pairs loaded on sync ahead of the adj stream
N_WARM = 7              # cold warmup matmuls bridging the DMA ramp

_PROG = None


def _build_program():
    from concourse import mybir, tile, bacc

    f32 = mybir.dt.float32
    f32r = mybir.dt.float32r
    bf16 = mybir.dt.bfloat16
    fp8 = mybir.dt.float8e4
    DR = mybir.MatmulPerfMode.DoubleRow
    RELU = mybir.ActivationFunctionType.Relu

    nc = bacc.Bacc(
        "TRN2",
        target_bir_lowering=False,
        debug=False,
        enable_asserts=False,
        num_devices=N_CORES,
    )
    # [p][round][jtile][i] — per partition each round's block is 16 KiB
    # contiguous, so every chunk DMA is clean per-partition runs.
    adj_d = nc.dram_tensor("adj_p", [P, N_ROUNDS, NJT, W], fp8, kind="ExternalInput")
    # x hi/lo interleaved by pair in consumption order:
    # [p][pair][slot lo|hi][jt-in-pair][d]
    x2_d = nc.dram_tensor("x2_p", [P, NPAIR, 2, 2, D], fp8, kind="ExternalInput")
    u_d = nc.dram_tensor("U", [D, D], bf16, kind="ExternalInput")
    # output [e, i_core] bf16, UNSCALED relu(agg@U); host transposes,
    # upcasts and divides by deg.
    out_d = nc.dram_tensor("out_t", [P, I_CORE], bf16, kind="ExternalOutput")

    # (pairs, dma engine name) per chunk, per round; consumption order.
    # Round 0 ramps with small chunks on sync (its first item, so the
    # first matmul's data lands ~3.3us in); pairs 12-15 of round 0 and
    # the second half of every later round ride the scalar queue, which
    # first carries the x head + U + x tail.
    CHUNKS = [
        [(1, "sync"), (1, "sync"), (2, "sync"), (4, "sync"), (4, "sync"), (4, "scalar")],
        [(8, "sync"), (8, "scalar")],
        [(8, "sync"), (8, "scalar")],
        [(8, "sync"), (8, "scalar")],
    ]

    with tile.TileContext(nc, trace_sim=False) as tc, ExitStack() as ctx:
        const_pool = ctx.enter_context(tc.tile_pool(name="const", bufs=1))
        adj_pool = ctx.enter_context(tc.tile_pool(name="adj", bufs=8))
        sum_pool = ctx.enter_context(tc.tile_pool(name="sum", bufs=2))
        out_pool = ctx.enter_context(tc.tile_pool(name="out", bufs=2))
        ps_c = ctx.enter_context(tc.tile_pool(name="ps_c", bufs=2, space="PSUM"))
        ps_o = ctx.enter_context(tc.tile_pool(name="ps_o", bufs=2, space="PSUM"))
        ps_w = ctx.enter_context(tc.tile_pool(name="ps_w", bufs=1, space="PSUM"))

        # --- warmup: dummy matmuls on a zeroed tile keep the PE busy
        # through the DMA ramp and flip the HAM clock-gate to 2.4 GHz
        # just as the first real chunk's semaphore fires. ---
        warm_sb = const_pool.tile([P, 2, W], fp8)
        nc.vector.memset(warm_sb[:], 0.0)
        warm_ps = ps_w.tile([P, W], f32, tag="warm")
        for _ in range(N_WARM):
            nc.tensor.matmul(
                warm_ps[:],
                warm_sb[:, :, 0:D],
                warm_sb[:],
                start=True,
                stop=True,
                perf_mode=DR,
            )

        # --- constant loads. The x head is one small tile and the tail
        # is PER-PAIR tiles/DMAs in consumption order, all on the scalar
        # queue (sync's first item is adj chunk 0). Fine granularity
        # matters: each pair's matmuls wait only on that pair's DMA, so
        # the scheduler's hoisted waits line up with arrival order
        # (a single 896 KiB tail DMA gated the whole stream at ~17us). ---
        x2h_sb = const_pool.tile([P, XS_HEAD, 2, 2, D], fp8)
        nc.scalar.dma_start(x2h_sb[:], x2_d[:, 0:XS_HEAD])
        u_sb = const_pool.tile([P, D], bf16)
        nc.scalar.dma_start(u_sb[:], u_d[:])
        # ~256 KiB chunks: big enough to sustain queue rate (64 KiB
        # DMAs collapsed the queue to ~66 GB/s), small enough that a
        # hoisted wait costs at most one chunk of slack.
        X_CHUNKS = [(XS_HEAD, 6), (6, 10), (10, NPAIR)]
        x_tiles = {}
        for lo, hi in X_CHUNKS:
            t = const_pool.tile([P, hi - lo, 2, 2, D], fp8, tag=f"xq{lo}")
            nc.scalar.dma_start(t[:], x2_d[:, lo:hi])
            for pt in range(lo, hi):
                x_tiles[pt] = (t, pt - lo)

        def x_slice(pt, slot):
            if pt < XS_HEAD:
                return x2h_sb[:, pt, slot]
            t, off = x_tiles[pt]
            return t[:, off, slot]

        def emit_tail(q, c_ps):
            """Tail of round q: one DVE copy of the accumulated PSUM to
            SBUF as bf16, one U-matmul over all 512 columns, then
            ReLU+bf16 cast and store. Only the last round splits the
            ACT+store into quarters alternating the two HWDGE queues so
            the end-of-kernel drain pipelines; early rounds store on
            scalar (its in-flight adj bytes keep flowing while the
            trigger waits on ACT)."""
            last = q == N_ROUNDS - 1
            split = 4 if last else 1
            ws = W // split
            o_ps = ps_o.tile([P, W], f32, tag="ops")
            c_sb = sum_pool.tile([P, W], bf16, tag="csb")
            nc.vector.tensor_copy(c_sb[:], c_ps[:])
            nc.tensor.matmul(o_ps[:], u_sb[:], c_sb[:], start=True, stop=True)
            for h in range(split):
                cs = slice(h * ws, (h + 1) * ws)
                out_sb = out_pool.tile([P, ws], bf16, tag=f"osb{split}{h}")
                nc.scalar.activation(out_sb[:], o_ps[:, cs], RELU)
                eng = (nc.scalar if h % 2 == 0 else nc.sync) if last else nc.scalar
                eng.dma_start(
                    out_d[:, q * W + h * ws : q * W + (h + 1) * ws], out_sb[:]
                )

        pending = None
        for q in range(N_ROUNDS):
            c_ps = ps_c.tile([P, W], f32, tag="c")
            chunk_list = CHUNKS[q]
            n_chunks = len(chunk_list)
            pt0 = 0
            for c, (cp, eng_name) in enumerate(chunk_list):
                adj_sb = adj_pool.tile([P, 2 * cp, W], fp8, tag="adj")
                getattr(nc, eng_name).dma_start(
                    adj_sb[:], adj_d[:, q, 2 * pt0 : 2 * (pt0 + cp), :]
                )
                for u in range(cp):
                    pt = pt0 + u
                    for slot in range(2):  # lo then hi
                        nc.tensor.matmul(
                            c_ps[:],
                            x_slice(pt, slot),
                            adj_sb[:, 2 * u : 2 * u + 2, :],
                            start=(c == 0 and u == 0 and slot == 0),
                            stop=(c == n_chunks - 1 and u == cp - 1 and slot == 1),
                            perf_mode=DR,
                        )
                pt0 += cp
                if pending is not None and c == 0:
                    emit_tail(*pending)
                    pending = None
            pending = (q, c_ps)
        emit_tail(*pending)

    nc.compile()
    return nc


def _get_program():
    global _PROG
    if _PROG is None:
        _PROG = _build_program()
    return _PROG


E4 = ml_dtypes.float8_e4m3


def _shard_inputs(x, adj_mat, U):
    # adj -> fp8 via bit trick: 0/1 exact (1.0 == 0x38 in e4m3).
    adj8 = (adj_mat != 0).astype(np.uint8) * np.uint8(0x38)
    x32 = np.asarray(x, dtype=np.float32)
    u16 = np.ascontiguousarray(U.astype(ml_dtypes.bfloat16))
    in_maps = []
    for c in range(N_CORES):
        b, half = c // 2, c % 2
        i0 = half * I_CORE
        a = adj8[b, :, i0 : i0 + I_CORE]  # [N, I_CORE] uint8
        # [t*128+p, q*512+i] -> [p, q, t, i]
        a = np.ascontiguousarray(
            a.reshape(NJT, P, N_ROUNDS, W).transpose(1, 2, 0, 3)
        ).view(E4)
        xb = x32[b]  # [N, D]
        xh = xb.astype(E4)
        xl = (xb - xh.astype(np.float32)).astype(E4)
        # [pt, jtp, p, d] -> [p, pt, slot, jtp, d]
        x2 = np.empty((P, NPAIR, 2, 2, D), dtype=E4)
        x2[:, :, 0] = xl.reshape(NPAIR, 2, P, D).transpose(2, 0, 1, 3)
        x2[:, :, 1] = xh.reshape(NPAIR, 2, P, D).transpose(2, 0, 1, 3)
        in_maps.append({"adj_p": a, "x2_p": x2, "U": u16})
    return in_maps


def _run(x, adj_mat, U, trace=False):
    from concourse.bass_utils import run_bass_kernel_spmd

    nc = _get_program()
    in_maps = _shard_inputs(x, adj_mat, U)
    res = run_bass_kernel_spmd(
        nc, in_maps, core_ids=list(range(N_CORES)), trace=trace
    )
    deg = np.asarray(adj_mat, dtype=np.float32).sum(axis=1)  # [B, N]
    out = np.empty((B, N, D), dtype=np.float32)
    for c in range(N_CORES):
        b, half = c // 2, c % 2
        i0 = half * I_CORE
        ot = res.results[c]["out_t"].astype(np.float32)  # [128 e, I_CORE]
        out[b, i0 : i0 + I_CORE, :] = (ot / deg[b, i0 : i0 + I_CORE][None, :]).T
    return out, res


def kernel(x, adj_mat, U):
    out, _ = _run(
        np.asarray(x, dtype=np.float32),
        np.asarray(adj_mat, dtype=np.float32),
        np.asarray(U, dtype=np.float32),
    )
    return out


# revision 19
# speedup vs baseline: 1.1490x; 1.0656x over previous
"""GNN message-passing ConvNet layer on 8 TRN2 NeuronCores (Bass/Tile).

Computes, for x [B=4, N=4096, D=128], adj_mat [B, N, N] (0/1 floats),
U [D, D]:
    mask = (adj_mat > 0)
    deg[b, i] = sum_j adj_mat[b, j, i]
    agg[b, i, :] = sum_j mask[b, j, i] * x[b, j, :]
    out = relu((agg @ U) / deg[..., None])

Sharding: core c handles batch c//2 and destination-node half c%2 (the
column slice adj[b, :, i0:i0+2048]) — no collectives, identical per-core
work.

Per-core kernel (v2 pipeline; the v1 at 55-59us lost ~10us to a HAM
re-throttle caused by the x_lo tail crawling in on the SWDGE queue, and
~8us to an end-of-kernel semaphore chain over ~55 tiles):
  - adj is 0/1 so it is packed host-side to float8e4 (exact) — 8 MiB per
    core instead of 32 MiB. DRAM layout [128p][round][jtile][i] makes
    every DMA fully contiguous per partition.
  - x is split host-side into an fp8 hi/lo pair (x ~= hi + lo at ~bf16
    accuracy), interleaved by 256-row pair in consumption order in ONE
    tensor, and loaded over the two HWDGE queues (head on sync, tail on
    scalar) so no matmul ever waits on the slow SWDGE path.
  - Per 256-row j-pair, two DoubleRow fp8 matmuls (lo then hi) stream
    the same adj tile and accumulate into a SINGLE PSUM bank — there is
    no separate A/B combine, no degree row, no reciprocal, no partition
    broadcast. relu(z)/deg == relu(z/deg) for deg>0, so the 1/deg
    column scale is applied on the host after the gather (deg is
    computed host-side from adj; the HW returns relu(agg @ U) only).
  - adj chunks alternate between the sync and scalar HWDGE queues
    (~175 GB/s each when both active, ~350 aggregate vs the 358 GB/s
    per-core HBM cap), in consumption order per queue.
  - DoubleRow contracts 256 rows/output-column at 1 col/cycle, so the
    agg stream is ~66k PE cycles (~27.5 us warm) — the PE floor for the
    hi+lo algorithm; the DMA floor (~9.5 MiB at ~350 GB/s) is ~27.5 us
    too, so the two overlap almost exactly.
  - A handful of warmup matmuls on a zeroed scratch tile cover the
    ~3.5 us DMA ramp and flip the HAM clock-gate to 2.4 GHz just before
    real data lands; the real stream then keeps the PE saturated so the
    gate never drops back.
  - Per-round tail (emitted one chunk into the next round): DVE copies
    the round's PSUM to SBUF as f32r, one U-matmul (stationary U f32r,
    moving the copied sum) lands relu-input in PSUM, ACT applies ReLU
    and casts to bf16, and the [e, i] tile is stored (host transposes,
    upcasts, and divides by deg). Rounds 0-2 store on the SWDGE queue;
    the final round is split in column halves across sync+scalar so the
    end-of-kernel drain pipelines.
"""

import os
import sys

for _p in ("/opt/trn_rl_repo",):
    if _p not in sys.path and os.path.isdir(_p):
        sys.path.insert(0, _p)

from contextlib import ExitStack

import numpy as np
import ml_dtypes

B, N, D = 4, 4096, 128
P = 128
N_CORES = 8
W = 512                 # destination columns per round (one PSUM bank)
I_CORE = N // 2         # destination columns per core
N_ROUNDS = I_CORE // W  # 4
NJT = N // P            # 32 j-tiles of 128 rows
NPAIR = NJT // 2        # 16 DoubleRow pairs of 256 rows
XS_HEAD = 2             # x pairs loaded on sync ahead of the adj stream
N_WARM = 9              # cold warmup matmuls bridging the DMA ramp

_PROG = None


def _build_program():
    from concourse import mybir, tile, bacc

    f32 = mybir.dt.float32
    f32r = mybir.dt.float32r
    bf16 = mybir.dt.bfloat16
    fp8 = mybir.dt.float8e4
    DR = mybir.MatmulPerfMode.DoubleRow
    RELU = mybir.ActivationFunctionType.Relu

    nc = bacc.Bacc(
        "TRN2",
        target_bir_lowering=False,
        debug=False,
        enable_asserts=False,
        num_devices=N_CORES,
    )
    # [p][round][jtile][i] — per partition each round's block is 16 KiB
    # contiguous, so every chunk DMA is clean per-partition runs.
    adj_d = nc.dram_tensor("adj_p", [P, N_ROUNDS, NJT, W], fp8, kind="ExternalInput")
    # x hi/lo interleaved by pair in consumption order:
    # [p][pair][slot lo|hi][jt-in-pair][d]
    x2_d = nc.dram_tensor("x2_p", [P, NPAIR, 2, 2, D], fp8, kind="ExternalInput")
    u_d = nc.dram_tensor("U", [D, D], bf16, kind="ExternalInput")
    # output [e, i_core] bf16, UNSCALED relu(agg@U); host transposes,
    # upcasts and divides by deg.
    out_d = nc.dram_tensor("out_t", [P, I_CORE], bf16, kind="ExternalOutput")

    # (pairs, dma engine name) per chunk, per round; consumption order.
    # Round 0 alternates modest chunks between the queues (both already
    # carry half of x ahead of it); later rounds split 8/8.
    CHUNKS = [
        [(2, "sync"), (2, "scalar"), (4, "sync"), (4, "scalar"), (4, "sync")],
        [(8, "sync"), (8, "scalar")],
        [(8, "sync"), (8, "scalar")],
        [(8, "sync"), (8, "scalar")],
    ]

    with tile.TileContext(nc, trace_sim=False) as tc, ExitStack() as ctx:
        const_pool = ctx.enter_context(tc.tile_pool(name="const", bufs=1))
        adj_pool = ctx.enter_context(tc.tile_pool(name="adj", bufs=8))
        sum_pool = ctx.enter_context(tc.tile_pool(name="sum", bufs=2))
        out_pool = ctx.enter_context(tc.tile_pool(name="out", bufs=2))
        ps_c = ctx.enter_context(tc.tile_pool(name="ps_c", bufs=2, space="PSUM"))
        ps_o = ctx.enter_context(tc.tile_pool(name="ps_o", bufs=2, space="PSUM"))
        ps_w = ctx.enter_context(tc.tile_pool(name="ps_w", bufs=1, space="PSUM"))

        # --- warmup: dummy matmuls on a zeroed tile keep the PE busy
        # through the DMA ramp and flip the HAM clock-gate to 2.4 GHz
        # just as the first real chunk's semaphore fires. ---
        warm_sb = const_pool.tile([P, 2, W], fp8)
        nc.vector.memset(warm_sb[:], 0.0)
        warm_ps = ps_w.tile([P, W], f32, tag="warm")
        for _ in range(N_WARM):
            nc.tensor.matmul(
                warm_ps[:],
                warm_sb[:, :, 0:D],
                warm_sb[:],
                start=True,
                stop=True,
                perf_mode=DR,
            )

        # --- constant loads. ALL of x lands before any adj chunk: the
        # Tile scheduler hoists DMA-completion waits onto the earliest
        # instructions its cost model believes are safe, so any x DMA
        # that really completes after the stream starts can gate the
        # whole stream. x is split half per HWDGE queue (512 KiB each,
        # first item on both queues), done by ~11us; the warmup matmuls
        # cover exactly that window. ---
        XH = NPAIR // 2
        x2a_sb = const_pool.tile([P, XH, 2, 2, D], fp8)
        nc.sync.dma_start(x2a_sb[:], x2_d[:, 0:XH])
        x2b_sb = const_pool.tile([P, NPAIR - XH, 2, 2, D], fp8)
        nc.scalar.dma_start(x2b_sb[:], x2_d[:, XH:])
        u_sb = const_pool.tile([P, D], bf16)
        nc.scalar.dma_start(u_sb[:], u_d[:])

        def x_slice(pt, slot):
            if pt < XH:
                return x2a_sb[:, pt, slot]
            return x2b_sb[:, pt - XH, slot]

        def emit_tail(q, c_ps):
            """Tail of round q: DVE copy of the accumulated PSUM to SBUF
            as bf16, U-matmul, ReLU+bf16 cast, store. For early rounds
            the whole chain runs at reduced priority so the scheduler
            slots the U-matmul a few DR matmuls past the round boundary
            (the DVE copy takes ~0.6us; placed too early it stalls the
            PE). The last round is split in halves across the queues so
            the end-of-kernel drain pipelines."""
            last = q == N_ROUNDS - 1
            split = 2 if last else 1
            ws = W // split
            o_ps = ps_o.tile([P, W], f32, tag="ops")
            with ExitStack() as pctx:
                if not last:
                    pctx.enter_context(tc.high_priority(offset=-24))
                for h in range(split):
                    cs = slice(h * ws, (h + 1) * ws)
                    c_sb = sum_pool.tile([P, ws], bf16, tag=f"csb{split}{h}")
                    nc.vector.tensor_copy(c_sb[:], c_ps[:, cs])
                    nc.tensor.matmul(
                        o_ps[:, cs], u_sb[:], c_sb[:], start=True, stop=True
                    )
                    out_sb = out_pool.tile([P, ws], bf16, tag=f"osb{split}{h}")
                    nc.scalar.activation(out_sb[:], o_ps[:, cs], RELU)
                    eng = (nc.scalar if h == 0 else nc.sync) if last else nc.scalar
                    eng.dma_start(
                        out_d[:, q * W + h * ws : q * W + (h + 1) * ws], out_sb[:]
                    )

        pending = None
        for q in range(N_ROUNDS):
            c_ps = ps_c.tile([P, W], f32, tag="c")
            chunk_list = CHUNKS[q]
            n_chunks = len(chunk_list)
            pt0 = 0
            for c, (cp, eng_name) in enumerate(chunk_list):
                adj_sb = adj_pool.tile([P, 2 * cp, W], fp8, tag="adj")
                getattr(nc, eng_name).dma_start(
                    adj_sb[:], adj_d[:, q, 2 * pt0 : 2 * (pt0 + cp), :]
                )
                for u in range(cp):
                    pt = pt0 + u
                    for slot in range(2):  # lo then hi
                        nc.tensor.matmul(
                            c_ps[:],
                            x_slice(pt, slot),
                            adj_sb[:, 2 * u : 2 * u + 2, :],
                            start=(c == 0 and u == 0 and slot == 0),
                            stop=(c == n_chunks - 1 and u == cp - 1 and slot == 1),
                            perf_mode=DR,
                        )
                pt0 += cp
                if pending is not None and c == 0:
                    emit_tail(*pending)
                    pending = None
            pending = (q, c_ps)
        emit_tail(*pending)

    nc.compile()
    return nc


def _get_program():
    global _PROG
    if _PROG is None:
        _PROG = _build_program()
    return _PROG


E4 = ml_dtypes.float8_e4m3


def _shard_inputs(x, adj_mat, U):
    # adj -> fp8 via bit trick: 0/1 exact (1.0 == 0x38 in e4m3).
    adj8 = (adj_mat != 0).astype(np.uint8) * np.uint8(0x38)
    x32 = np.asarray(x, dtype=np.float32)
    u16 = np.ascontiguousarray(U.astype(ml_dtypes.bfloat16))
    in_maps = []
    for c in range(N_CORES):
        b, half = c // 2, c % 2
        i0 = half * I_CORE
        a = adj8[b, :, i0 : i0 + I_CORE]  # [N, I_CORE] uint8
        # [t*128+p, q*512+i] -> [p, q, t, i]
        a = np.ascontiguousarray(
            a.reshape(NJT, P, N_ROUNDS, W).transpose(1, 2, 0, 3)
        ).view(E4)
        xb = x32[b]  # [N, D]
        xh = xb.astype(E4)
        xl = (xb - xh.astype(np.float32)).astype(E4)
        # [pt, jtp, p, d] -> [p, pt, slot, jtp, d]
        x2 = np.empty((P, NPAIR, 2, 2, D), dtype=E4)
        x2[:, :, 0] = xl.reshape(NPAIR, 2, P, D).transpose(2, 0, 1, 3)
        x2[:, :, 1] = xh.reshape(NPAIR, 2, P, D).transpose(2, 0, 1, 3)
        in_maps.append({"adj_p": a, "x2_p": x2, "U": u16})
    return in_maps


def _run(x, adj_mat, U, trace=False):
    from concourse.bass_utils import run_bass_kernel_spmd

    nc = _get_program()
    in_maps = _shard_inputs(x, adj_mat, U)
    res = run_bass_kernel_spmd(
        nc, in_maps, core_ids=list(range(N_CORES)), trace=trace
    )
    deg = np.asarray(adj_mat, dtype=np.float32).sum(axis=1)  # [B, N]
    out = np.empty((B, N, D), dtype=np.float32)
    for c in range(N_CORES):
        b, half = c // 2, c % 2
        i0 = half * I_CORE
        ot = res.results[c]["out_t"].astype(np.float32)  # [128 e, I_CORE]
        out[b, i0 : i0 + I_CORE, :] = (ot / deg[b, i0 : i0 + I_CORE][None, :]).T
    return out, res


def kernel(x, adj_mat, U):
    out, _ = _run(
        np.asarray(x, dtype=np.float32),
        np.asarray(adj_mat, dtype=np.float32),
        np.asarray(U, dtype=np.float32),
    )
    return out
